# revision 6
# baseline (speedup 1.0000x reference)
"""Trainium2 Bass kernel for a CGNS block (GNN message passing).

Math: the reference builds A = a a^T + I (rank-1 + identity), L = D^-1/2 A D^-1/2,
then out = relu(BN(conv1x1(cat[x@A, (L@x^T)^T]))).  Exploiting the rank-1
structure, with a = relu(tanh(w)), S = sum(a), d_n = 1/sqrt(a_n*S + 1),
u = d*a, s0 = x@a, s1 = x@u, the whole block collapses to

  y[:, n] = W1~ x[:, n] + d2[n] * (W2~ x[:, n]) + a[n] v1 + u[n] v2 + b~
  out     = relu(y)

where W~ are the BN-folded conv weights, v1 = W1~ s0, v2 = W2~ s1.  No [N,N]
matrix is ever materialized.

Sharding: 8 cores; core i handles batch b = i//2, half h = i%2 of the N=4096
node dim (2048 columns each).  Each core reads the full x[b] once in
transposed layout (for the s0/s1 reduction, which needs all of N) and its own
half in natural layout (for the main matmuls).  n-chunks are rolled per-core
so that chunks 0..15 are always the core's own half -> identical SPMD program.

v2: all matmul-path data is bf16 (4x PE rate vs fp32's 4-cycles/row, half the
HBM bytes; tolerance is 2e-2 so bf16's ~0.4% is fine).  The a/u row layout
(xa partitions 64/65) comes from two PE transposes of the column-layout
values instead of a second scalar-engine chain - this removes two 1.28us
ACT_TABLE_LOADs and a ~5us serial dependency that gated the main matmuls.
The epilogue reads both STT operands straight from PSUM and alternates
Vector/GpSimd so neither engine serializes the tail.
"""

import numpy as np

import concourse.bacc as bacc
import concourse.bass as bass
import concourse.tile as tile
from concourse import masks, mybir

FP = mybir.dt.float32
BF = mybir.dt.bfloat16
B, C, N = 4, 64, 4096
NH = N // 2          # columns per core
JH = NH // 128       # 16 chunks per core half
JF = N // 128        # 32 chunks full N
BN_EPS = 1e-5


def build_nc():
    # Bacc (not raw Bass): its compile() pipeline legalizes TRN2's
    # one-wait-per-instruction constraint (move_matmul_waits_to_ldweights,
    # generate_event_semaphores) which Tile-emitted multi-waits require.
    nc = bacc.Bacc()
    AF = mybir.ActivationFunctionType
    OP = mybir.AluOpType

    # DRAM I/O (per-core shards supplied via in_maps)
    xt = nc.dram_tensor("xt", [128, JF, C], BF, kind="ExternalInput")
    xh = nc.dram_tensor("xh", [C, NH], BF, kind="ExternalInput")
    wcol = nc.dram_tensor("wcol", [128, 32], FP, kind="ExternalInput")
    wv = nc.dram_tensor("wv", [C, 2 * C], BF, kind="ExternalInput")
    brow3 = nc.dram_tensor("brow3", [3, 2 * C], BF, kind="ExternalInput")
    out = nc.dram_tensor("out", [128, JH, C], BF, kind="ExternalOutput")

    with tile.TileContext(nc) as tc:
        with (
            tc.tile_pool(name="sb", bufs=1) as sb,
            tc.tile_pool(name="ps", bufs=1, space="PSUM") as ps,
        ):
            # SBUF tiles
            xt_sb = sb.tile([128, JF, C], BF, name="xt_sb")
            xa = sb.tile([67, NH], BF, name="xa")       # x half + a/u/ones rows
            wcol_sb = sb.tile([128, 32], FP, name="wcol_sb")
            wAB = sb.tile([67, 2 * C], BF, name="wAB")  # [W1~T|W2~T] + v/b~ rows
            ones = sb.tile([128, 128], FP, name="ones")
            ident = sb.tile([128, 128], BF, name="ident")
            ones16 = sb.tile([JH, 128], BF, name="ones16")
            rowsb = sb.tile([JH, 2 * 128], BF, name="rowsb")
            tcol = sb.tile([128, 32], FP, name="tcol")
            acol = sb.tile([128, 32], FP, name="acol")
            ttile = sb.tile([128, 32], FP, name="ttile")
            dcol = sb.tile([128, 32], FP, name="dcol")
            ucol = sb.tile([128, 32], FP, name="ucol")
            d2col = sb.tile([128, 32], FP, name="d2col")
            apart = sb.tile([128, 1], FP, name="apart")
            sS = sb.tile([128, 1], FP, name="sS")
            au = sb.tile([128, 2 * 32], BF, name="au")  # a/u interleaved, bf16
            s01 = sb.tile([C, 2], BF, name="s01")
            vtmp = sb.tile([1, 2 * C], BF, name="vtmp")
            y1 = sb.tile([128, JH * C], FP, name="y1")
            qd2 = sb.tile([128, JH * C], BF, name="qd2")
            yo = sb.tile([128, JH * C], BF, name="yo")

            # PSUM tiles (each padded to a bank; 8 total <= 8 banks)
            p_sm = ps.tile([128, 1], FP, name="p_sm")
            p_s = ps.tile([C, 2], FP, name="p_s")
            p_v = ps.tile([1, 2 * C], FP, name="p_v")
            p_t = ps.tile([JH, 2 * 128], BF, name="p_t")
            p_yq = [ps.tile([128, 512], FP, name=f"p_yq_{g}") for g in range(4)]

            # ---- DMAs in.  wcol first on SP (it gates the scalar chain);
            # xt split across the Activation and SP HWDGE queues; xh/wv/brow3
            # later on SP (needed only by the main matmuls).
            nc.sync.dma_start(wcol_sb[:], wcol[:])
            nc.scalar.dma_start(xt_sb[:, 0:8, :], xt[:, 0:8, :])
            nc.scalar.dma_start(xt_sb[:, 8:16, :], xt[:, 8:16, :])
            nc.sync.dma_start(xt_sb[:, 16:24, :], xt[:, 16:24, :])
            nc.sync.dma_start(xt_sb[:, 24:32, :], xt[:, 24:32, :])
            nc.sync.dma_start(wAB[0:C, :], wv[:])
            nc.sync.dma_start(wAB[64:67, :], brow3[:])
            nc.sync.dma_start(xa[0:C, 0:1024], xh[:, 0:1024])
            nc.sync.dma_start(xa[0:C, 1024:2048], xh[:, 1024:2048])

            # constants: fp32 ones (S broadcast), bf16 identity (PE transpose),
            # bf16 ones row source for xa row 66 (DMA: engine writes at
            # partition 64+ hang HW, DMA has no partition restrictions).
            nc.vector.memset(ones[:], 1.0)
            nc.gpsimd.memset(ones16[:], 1.0)
            nc.gpsimd.dma_start(xa[66:67, :], ones16[:])
            masks.make_identity(nc, ident[:])

            # ---- scalar/vector front-end (column layout, fp32):
            # a = relu(tanh(w)) with the partial row-sum fused via accum_out;
            # S broadcast to all partitions via ones-matmul; t = a*S + 1;
            # d2 = 1/t (vector); d = sqrt(d2) (scalar; its ACT table load
            # hides behind the vector ops); u = d*a.
            nc.scalar.activation(tcol[:], wcol_sb[:], AF.Tanh)
            nc.scalar.activation(acol[:], tcol[:], AF.Relu, accum_out=apart[:])
            nc.tensor.matmul(p_sm[:], ones[:], apart[:], start=True, stop=True)
            au_v = au[:].rearrange("p (k t) -> p k t", t=2)
            nc.vector.tensor_copy(au_v[:, :, 0], acol[:])
            nc.vector.tensor_copy(sS[:], p_sm[:])
            nc.vector.tensor_scalar(
                ttile[:], acol[:], sS[:], 1.0, op0=OP.mult, op1=OP.add
            )
            nc.vector.reciprocal(d2col[:], ttile[:])
            nc.scalar.sqrt(dcol[:], d2col[:])
            nc.vector.tensor_mul(ucol[:], dcol[:], acol[:])
            nc.vector.tensor_copy(au_v[:, :, 1], ucol[:])

            # ---- a/u row layout via PE transpose of the own-half columns
            # (chunks 0..15 are the core's own half by construction).
            nc.tensor.transpose(p_t[:, 0:128], au_v[:, 0:JH, 0], ident[:])
            nc.tensor.transpose(p_t[:, 128:256], au_v[:, 0:JH, 1], ident[:])
            nc.vector.tensor_copy(rowsb[:], p_t[:])
            nc.gpsimd.dma_start(xa[64:65, :], rowsb[:, 0:128])
            nc.gpsimd.dma_start(xa[65:66, :], rowsb[:, 128:256])

            # ---- s0/s1 reduction over full N (PE, accumulate in PSUM) ----
            for j in range(JF):
                nc.tensor.matmul(
                    p_s[:],
                    xt_sb[:, j, :],
                    au[:, 2 * j : 2 * j + 2],
                    start=(j == 0),
                    stop=(j == JF - 1),
                )
            nc.vector.tensor_copy(s01[:], p_s[:])

            # v1/v2 on partition 0 side by side, one evacuation, one DMA into
            # wAB rows 64/65 (engine writes at partition 64+ hang HW).
            nc.tensor.matmul(
                p_v[0:1, 0:C], s01[:, 0:1], wAB[0:C, 0:C], start=True, stop=True
            )
            nc.tensor.matmul(
                p_v[0:1, C : 2 * C], s01[:, 1:2], wAB[0:C, C : 2 * C],
                start=True, stop=True,
            )
            nc.vector.tensor_copy(vtmp[:], p_v[:])
            nc.sync.dma_start(
                wAB[64:66, 0:C],
                vtmp[:].rearrange("p (r c) -> p r c", c=C),
            )

            # ---- main matmuls: one [67,128]x[67,128] mm per chunk.
            # out columns 0:64 = y1 (conv1 + rank-2 + bias), 64:128 = q (conv2)
            for j in range(JH):
                grp, jj = divmod(j, 4)
                nc.tensor.matmul(
                    p_yq[grp][:, 128 * jj : 128 * (jj + 1)],
                    xa[:, 128 * j : 128 * (j + 1)],
                    wAB[:],
                    start=True, stop=True,
                )

            # ---- epilogue: yo = relu(q * d2 + y1), group-wide [128,256] ops.
            # Engine constraints: only one PSUM operand per DVE op, and
            # GpSimd can't touch PSUM at all.  So: Scalar evacuates y1,
            # Vector does q*d2 (PSUM read; d2 broadcast per-chunk along the
            # free dim), GpSimd adds the two SBUF halves, relus alternate.
            for g in range(4):
                gs = slice(256 * g, 256 * (g + 1))
                nc.scalar.copy(
                    y1[:, gs].rearrange("p (j c) -> p j c", c=C),
                    p_yq[g][:].rearrange("p (j c) -> p j c", c=2 * C)[:, :, 0:C],
                )
                nc.vector.tensor_tensor(
                    qd2[:, gs].rearrange("p (j c) -> p j c", c=C),
                    p_yq[g][:].rearrange("p (j c) -> p j c", c=2 * C)[:, :, C : 2 * C],
                    d2col[:, 4 * g : 4 * (g + 1), None].broadcast_to((128, 4, C)),
                    op=OP.mult,
                )
                nc.gpsimd.tensor_tensor(
                    yo[:, gs], qd2[:, gs], y1[:, gs], op=OP.add
                )
                eng = nc.vector if g % 2 == 0 else nc.gpsimd
                eng.tensor_scalar_max(yo[:, gs], yo[:, gs], 0.0)
                nc.sync.dma_start(
                    out[:, 4 * g : 4 * (g + 1), :],
                    yo[:, gs].rearrange("p (j c) -> p j c", c=C),
                )
    nc.compile()
    return nc


def make_in_maps(x, w, conv_w, conv_b, bn_gamma, bn_beta, bn_mean, bn_var):
    import ml_dtypes

    bf16 = ml_dtypes.bfloat16
    x = np.asarray(x, np.float32)
    w = np.asarray(w, np.float32)
    conv_w = np.asarray(conv_w, np.float32)
    conv_b = np.asarray(conv_b, np.float32)
    bn_gamma = np.asarray(bn_gamma, np.float32)
    bn_beta = np.asarray(bn_beta, np.float32)
    bn_mean = np.asarray(bn_mean, np.float32)
    bn_var = np.asarray(bn_var, np.float32)

    scale = bn_gamma / np.sqrt(bn_var + BN_EPS)
    wmat = conv_w * scale[:, None]                       # [64, 128] BN-folded
    w1t = np.ascontiguousarray(wmat[:, :C].T)            # [c, o]
    w2t = np.ascontiguousarray(wmat[:, C:].T)
    wv = np.ascontiguousarray(
        np.concatenate([w1t, w2t], axis=1).astype(bf16)
    )
    brow3 = np.zeros((3, 2 * C), np.float32)
    brow3[2, :C] = conv_b * scale + bn_beta - bn_mean * scale
    brow3 = brow3.astype(bf16)

    in_maps = []
    for i in range(8):
        b, h = divmod(i, 2)
        xb = x[b, :, :, 0]                               # [64, 4096]
        order = np.roll(np.arange(JF), -JH * h)          # own half first
        xt_jpc = np.ascontiguousarray(xb.T).reshape(JF, 128, C)
        xt_pjc = np.ascontiguousarray(
            xt_jpc[order].transpose(1, 0, 2).astype(bf16)
        )
        xhb = np.ascontiguousarray(xb[:, NH * h : NH * (h + 1)].astype(bf16))
        wcol = np.ascontiguousarray(w[b].reshape(JF, 128).T[:, order])
        in_maps.append(
            {
                "xt": xt_pjc,
                "xh": xhb,
                "wcol": wcol,
                "wv": wv,
                "brow3": brow3,
            }
        )
    return in_maps


def assemble_out(results):
    out = np.empty((B, C, N), np.float32)
    for i in range(8):
        b, h = divmod(i, 2)
        blk = np.asarray(results[i]["out"]).astype(np.float32)  # [128, 16, 64]
        y_half = blk.transpose(1, 0, 2).reshape(NH, C)   # row = 128*j + p
        out[b, :, NH * h : NH * (h + 1)] = y_half.T
    return out[..., None]


_NC = None


def kernel(**inputs):
    global _NC
    from concourse.bass_utils import run_bass_kernel_spmd

    if _NC is None:
        _NC = build_nc()
    in_maps = make_in_maps(**inputs)
    res = run_bass_kernel_spmd(_NC, in_maps, list(range(8)))
    return assemble_out(res.results)


# revision 12
# speedup vs baseline: 1.2086x; 1.2086x over previous
"""Trainium2 Bass kernel for a CGNS block (GNN message passing).

Math: the reference builds A = a a^T + I (rank-1 + identity), L = D^-1/2 A D^-1/2,
then out = relu(BN(conv1x1(cat[x@A, (L@x^T)^T]))).  Exploiting the rank-1
structure, with a = relu(tanh(w)), S = sum(a), d_n = 1/sqrt(a_n*S + 1),
u = d*a, s0 = x@a, s1 = x@u, the whole block collapses to

  y[:, n] = W1~ x[:, n] + d2[n] * (W2~ x[:, n]) + a[n] v1 + u[n] v2 + b~
  out     = relu(y)

where W~ are the BN-folded conv weights, v1 = W1~ s0, v2 = W2~ s1.  No [N,N]
matrix is ever materialized.

Sharding: 8 cores; core i handles batch b = i//2, half h = i%2 of the N=4096
node dim (2048 columns each).  Each core reads the full x[b] once in
transposed layout (for the s0/s1 reduction, which needs all of N) and its own
half in natural layout (for the main matmuls).  n-chunks are rolled per-core
so that chunks 0..15 are always the core's own half -> identical SPMD program.

v3 structure (all matmul-path data bf16; tolerance is 2e-2 so bf16's ~0.4%
noise is fine and PE runs 4x faster than fp32's 4-cycles/row):
 - main mm per chunk = mm_a ([65,128] x/ones stationary vs [W1~T|W2~T]+[b~|0]
   moving; depends only on early DMAs) + two K=1 rank-1 accumulations
   (a-row/u-row chunk from the PE-transpose staging tile at partition j,
   times v1/v2 slices of vtmp on partition 0).  This removes BOTH partition-
   64+ row DMAs and the wAB v-row DMA roundtrip that stalled the PE.
 - a/u rows come from two PE transposes of the column-layout values, not a
   second scalar-engine chain (no extra ACT_TABLE_LOADs).
 - epilogue: Vector does qd2 = q(PSUM)*d2 (d2 broadcast per-chunk along the
   free dim) then yo = y1(PSUM) + qd2(SBUF) - each TT has exactly one PSUM
   operand.  Scalar does only the relus.  GpSimd (slow ucode engine) does
   no elementwise work at all.
"""

import numpy as np

import concourse.bacc as bacc
import concourse.bass as bass
import concourse.tile as tile
from concourse import masks, mybir

FP = mybir.dt.float32
BF = mybir.dt.bfloat16
B, C, N = 4, 64, 4096
NH = N // 2          # columns per core
JH = NH // 128       # 16 chunks per core half
JF = N // 128        # 32 chunks full N
BN_EPS = 1e-5


def build_nc():
    # Bacc (not raw Bass): its compile() pipeline legalizes TRN2's
    # one-wait-per-instruction constraint (move_matmul_waits_to_ldweights,
    # generate_event_semaphores) which Tile-emitted multi-waits require.
    nc = bacc.Bacc()
    AF = mybir.ActivationFunctionType
    OP = mybir.AluOpType

    # DRAM I/O (per-core shards supplied via in_maps)
    xt = nc.dram_tensor("xt", [128, JF, C], BF, kind="ExternalInput")
    xh = nc.dram_tensor("xh", [C, NH], BF, kind="ExternalInput")
    wcol = nc.dram_tensor("wcol", [128, 32], FP, kind="ExternalInput")
    wv = nc.dram_tensor("wv", [C, 2 * C], BF, kind="ExternalInput")
    brow1 = nc.dram_tensor("brow1", [1, 2 * C], BF, kind="ExternalInput")
    out = nc.dram_tensor("out", [128, JH, C], BF, kind="ExternalOutput")

    with tile.TileContext(nc) as tc:
        with (
            tc.tile_pool(name="sb", bufs=1) as sb,
            tc.tile_pool(name="ps", bufs=1, space="PSUM") as ps,
        ):
            # SBUF tiles
            xt_sb = sb.tile([128, JF, C], BF, name="xt_sb")
            xa = sb.tile([65, NH], BF, name="xa")        # x half + ones row
            wcol_sb = sb.tile([128, 32], FP, name="wcol_sb")
            wAB = sb.tile([65, 2 * C], BF, name="wAB")   # [W1~T|W2~T] + [b~|0]
            ones = sb.tile([128, 128], FP, name="ones")
            ident = sb.tile([128, 128], BF, name="ident")
            ones16 = sb.tile([JH, 128], BF, name="ones16")
            rowsb = sb.tile([JH, 2 * 128], BF, name="rowsb")  # a/u rows
            tcol = sb.tile([128, 32], FP, name="tcol")
            acol = sb.tile([128, 32], FP, name="acol")
            ttile = sb.tile([128, 32], FP, name="ttile")
            dcol = sb.tile([128, 32], FP, name="dcol")
            d2col = sb.tile([128, 32], FP, name="d2col")
            apart = sb.tile([128, 1], FP, name="apart")
            sS = sb.tile([128, 1], FP, name="sS")
            au = sb.tile([128, 2 * 32], BF, name="au")   # a/u interleaved
            s01a = sb.tile([C, 2], BF, name="s01a")      # [s0 | 0]
            s01b = sb.tile([C, 2], BF, name="s01b")      # [0 | s1]
            aurow2 = sb.tile([2, NH], BF, name="aurow2")  # a row / u row
            vvt = sb.tile([2, C], BF, name="vvt")        # v1 / v2 rows
            qd2 = sb.tile([128, JH * C], BF, name="qd2")
            yo = sb.tile([128, JH * C], BF, name="yo")

            # PSUM tiles (each padded to a bank; 8 total <= 8 banks)
            p_sm = ps.tile([128, 1], FP, name="p_sm")
            p_s = ps.tile([C, 2], FP, name="p_s")
            p_v = ps.tile([2, C], FP, name="p_v")
            p_t = ps.tile([JH, 2 * 128], BF, name="p_t")
            p_yq = [ps.tile([128, 512], FP, name=f"p_yq_{g}") for g in range(4)]

            # ---- DMAs in.  wcol first on SP (it gates the scalar chain),
            # then xh/wv/brow1 (gates of mm_a).  xt spread over the
            # Activation and Vector HWDGE queues, which are idle early.
            nc.sync.dma_start(wcol_sb[:], wcol[:])
            nc.scalar.dma_start(xt_sb[:, 0:8, :], xt[:, 0:8, :])
            nc.scalar.dma_start(xt_sb[:, 8:16, :], xt[:, 8:16, :])
            nc.gpsimd.dma_start(xt_sb[:, 16:24, :], xt[:, 16:24, :])
            nc.gpsimd.dma_start(xt_sb[:, 24:32, :], xt[:, 24:32, :])
            nc.sync.dma_start(xa[0:C, 0:1024], xh[:, 0:1024])
            nc.sync.dma_start(xa[0:C, 1024:2048], xh[:, 1024:2048])
            nc.sync.dma_start(wAB[0:C, :], wv[:])
            nc.sync.dma_start(wAB[64:65, :], brow1[:])

            # constants: fp32 ones (S broadcast), bf16 identity (PE
            # transpose), bf16 ones row -> xa row 64 via DMA (engine writes
            # at partition 64+ hang HW; DMA has no partition restrictions).
            nc.vector.memset(ones[:], 1.0)
            nc.gpsimd.memset(ones16[:], 1.0)
            nc.gpsimd.dma_start(xa[64:65, :], ones16[:])
            masks.make_identity(nc, ident[:])
            nc.vector.memset(s01a[:], 0.0)
            nc.vector.memset(s01b[:], 0.0)

            # ---- scalar/vector front-end (column layout, fp32):
            # a = relu(tanh(w)) with the partial row-sum fused via accum_out;
            # S broadcast to all partitions via ones-matmul; t = a*S + 1;
            # d2 = 1/t (vector); d = sqrt(d2) (scalar; its ACT table load
            # hides behind the vector ops); u = d*a written straight into
            # the bf16 interleaved a/u tile.
            nc.scalar.activation(tcol[:], wcol_sb[:], AF.Tanh)
            nc.scalar.activation(acol[:], tcol[:], AF.Relu, accum_out=apart[:])
            nc.tensor.matmul(p_sm[:], ones[:], apart[:], start=True, stop=True)
            au_v = au[:].rearrange("p (k t) -> p k t", t=2)
            nc.vector.tensor_copy(au_v[:, :, 0], acol[:])
            nc.vector.tensor_copy(sS[:], p_sm[:])
            nc.vector.tensor_scalar(
                ttile[:], acol[:], sS[:], 1.0, op0=OP.mult, op1=OP.add
            )
            nc.vector.reciprocal(d2col[:], ttile[:])
            nc.scalar.sqrt(dcol[:], d2col[:])
            nc.vector.tensor_mul(au_v[:, :, 1], dcol[:], acol[:])

            # ---- a/u row layout via PE transpose of the own-half columns
            # (chunks 0..15 are the core's own half by construction), then
            # one evacuation into the SBUF staging tile mm_b reads from.
            nc.tensor.transpose(p_t[:, 0:128], au_v[:, 0:JH, 0], ident[:])

            # mm_a for groups 0/1 squeezed into the PE's wait-for-u window.
            # One start=True per PSUM bank (jj==0): start marks the whole 2KB
            # zero-region pending-zero, so a second start on the same bank
            # would turn later accumulations into overwrites.
            def mm_a(j):
                grp, jj = divmod(j, 4)
                nc.tensor.matmul(
                    p_yq[grp][:, 128 * jj : 128 * (jj + 1)],
                    xa[:, 128 * j : 128 * (j + 1)],
                    wAB[:],
                    start=(jj == 0), stop=False,
                    skip_group_check=True,
                )

            for j in range(8):
                mm_a(j)

            nc.tensor.transpose(p_t[:, 128:256], au_v[:, 0:JH, 1], ident[:])
            nc.vector.tensor_copy(rowsb[:], p_t[:])
            # flatten the [16,128] row-staging into true [1, 2048] rows
            # (partition-crossing, so DMA) - off the PE critical path.
            nc.gpsimd.dma_start(aurow2[0:1, :], rowsb[:, 0:128])
            nc.gpsimd.dma_start(aurow2[1:2, :], rowsb[:, 128:256])

            # ---- s0/s1 reduction over full N (PE, accumulate in PSUM) ----
            for j in range(JF):
                nc.tensor.matmul(
                    p_s[:],
                    xt_sb[:, j, :],
                    au[:, 2 * j : 2 * j + 2],
                    start=(j == 0),
                    stop=(j == JF - 1),
                )
            nc.vector.tensor_copy(s01a[:, 0:1], p_s[:, 0:1])
            nc.vector.tensor_copy(s01b[:, 1:2], p_s[:, 1:2])

            for j in range(8, JH):
                mm_a(j)

            # v1/v2 stacked on partitions 0/1 via zero-padded stationaries:
            # [s0|0]^T W1~T -> [v1;0], [0|s1]^T W2~T -> [0;v2], accumulated.
            nc.tensor.matmul(
                p_v[:], s01a[:], wAB[0:C, 0:C], start=True, stop=False,
                skip_group_check=True,
            )
            nc.tensor.matmul(
                p_v[:], s01b[:], wAB[0:C, C : 2 * C], start=False, stop=True,
                skip_group_check=True,
            )
            nc.vector.tensor_copy(vvt[:], p_v[:])

            # ---- rank-2 term: one K=2 accumulation per chunk into the y1
            # columns: a[128j+m]*v1[c] + u[128j+m]*v2[c].
            for j in range(JH):
                grp, jj = divmod(j, 4)
                nc.tensor.matmul(
                    p_yq[grp][:, 128 * jj : 128 * jj + C],
                    aurow2[:, 128 * j : 128 * (j + 1)],
                    vvt[:],
                    start=False, stop=(jj == 3), skip_group_check=True,
                )

            # ---- epilogue: yo = relu(q*d2 + y1), group-wide [128,256] ops,
            # one PSUM operand per TT: Vector qd2 = q*d2 (right after mm_a of
            # the group), then yo = y1(PSUM) + qd2; relu on Scalar.
            for g in range(4):
                gs = slice(256 * g, 256 * (g + 1))
                nc.vector.tensor_tensor(
                    qd2[:, gs].rearrange("p (j c) -> p j c", c=C),
                    p_yq[g][:].rearrange("p (j c) -> p j c", c=2 * C)[:, :, C : 2 * C],
                    d2col[:, 4 * g : 4 * (g + 1), None].broadcast_to((128, 4, C)),
                    op=OP.mult,
                )
            for g in range(4):
                gs = slice(256 * g, 256 * (g + 1))
                nc.vector.tensor_tensor(
                    yo[:, gs].rearrange("p (j c) -> p j c", c=C),
                    p_yq[g][:].rearrange("p (j c) -> p j c", c=2 * C)[:, :, 0:C],
                    qd2[:, gs].rearrange("p (j c) -> p j c", c=C),
                    op=OP.add,
                )
                nc.scalar.activation(yo[:, gs], yo[:, gs], AF.Relu)
                nc.sync.dma_start(
                    out[:, 4 * g : 4 * (g + 1), :],
                    yo[:, gs].rearrange("p (j c) -> p j c", c=C),
                )
    nc.compile()
    return nc


def make_in_maps(x, w, conv_w, conv_b, bn_gamma, bn_beta, bn_mean, bn_var):
    import ml_dtypes

    bf16 = ml_dtypes.bfloat16
    x = np.asarray(x, np.float32)
    w = np.asarray(w, np.float32)
    conv_w = np.asarray(conv_w, np.float32)
    conv_b = np.asarray(conv_b, np.float32)
    bn_gamma = np.asarray(bn_gamma, np.float32)
    bn_beta = np.asarray(bn_beta, np.float32)
    bn_mean = np.asarray(bn_mean, np.float32)
    bn_var = np.asarray(bn_var, np.float32)

    scale = bn_gamma / np.sqrt(bn_var + BN_EPS)
    wmat = conv_w * scale[:, None]                       # [64, 128] BN-folded
    w1t = np.ascontiguousarray(wmat[:, :C].T)            # [c, o]
    w2t = np.ascontiguousarray(wmat[:, C:].T)
    wv = np.ascontiguousarray(
        np.concatenate([w1t, w2t], axis=1).astype(bf16)
    )
    brow1 = np.zeros((1, 2 * C), np.float32)
    brow1[0, :C] = conv_b * scale + bn_beta - bn_mean * scale
    brow1 = brow1.astype(bf16)

    in_maps = []
    for i in range(8):
        b, h = divmod(i, 2)
        xb = x[b, :, :, 0]                               # [64, 4096]
        order = np.roll(np.arange(JF), -JH * h)          # own half first
        xt_jpc = np.ascontiguousarray(xb.T).reshape(JF, 128, C)
        xt_pjc = np.ascontiguousarray(
            xt_jpc[order].transpose(1, 0, 2).astype(bf16)
        )
        xhb = np.ascontiguousarray(xb[:, NH * h : NH * (h + 1)].astype(bf16))
        wcol = np.ascontiguousarray(w[b].reshape(JF, 128).T[:, order])
        in_maps.append(
            {
                "xt": xt_pjc,
                "xh": xhb,
                "wcol": wcol,
                "wv": wv,
                "brow1": brow1,
            }
        )
    return in_maps


def assemble_out(results):
    out = np.empty((B, C, N), np.float32)
    for i in range(8):
        b, h = divmod(i, 2)
        blk = np.asarray(results[i]["out"]).astype(np.float32)  # [128, 16, 64]
        y_half = blk.transpose(1, 0, 2).reshape(NH, C)   # row = 128*j + p
        out[b, :, NH * h : NH * (h + 1)] = y_half.T
    return out[..., None]


_NC = None


def kernel(**inputs):
    global _NC
    from concourse.bass_utils import run_bass_kernel_spmd

    if _NC is None:
        _NC = build_nc()
    in_maps = make_in_maps(**inputs)
    res = run_bass_kernel_spmd(_NC, in_maps, list(range(8)))
    return assemble_out(res.results)
